# revision 1
# baseline (speedup 1.0000x reference)
"""CANLayer (two attention convs + linear, relu) on 8 trn2 NeuronCores.

Strategy: shard edges by target-node range (6250 rows/core, no collectives).
Per conv, a node table [xm | s_src | s_tgt] lives in HBM (built on device by
PE); per edge we dma_gather the source row (xm[j], s_src[j]) and the target's
s_tgt from a per-core local table, compute alpha = elu(s_src+s_tgt)*v on
DVE/ACT, scale the message, and dma_scatter_add (CCE f32) into a local
accumulator. The scatter uses 5 rank-images (row = i_loc + 6250*(rank%5)) so
every scatter call has unique rows (the CCE RMW loses updates on duplicate
rows within a call). Epilogue reduces the images, adds x@w_lin*EPS, applies
relu.
"""
import sys
import numpy as np

for _p in ('/opt/trn_rl_repo',):
    if _p not in sys.path:
        sys.path.insert(0, _p)

import ml_dtypes

bfloat16 = ml_dtypes.bfloat16

N = 50000
E = 800000
C = 64
NCORES = 8
NLOC = N // NCORES            # 6250
EPS = 1 + 1e-06

NT = 391                       # table tiles of 128 rows
RPAD = NT * 128                # 50048
NTL = 49                       # local tiles
LPAD = NTL * 128               # 6272
CH = 4096                      # edges per chunk (max per dma_gather call)
KIMG = 5                       # scatter rank images (separate acc tensors)
GARBAGE = LPAD                 # 6272: dedicated garbage row in each image
ACC_ROWS = 6400                # per-image rows (>= GARBAGE+1)
DENSE_T = 4200                 # leading rank groups >= this become dense (one
                               # slot per target; no i-gather, accum-DMA scatter)


def _round128(x):
    return (x + 127) // 128 * 128


def _prep_conv_core(indices, values, core):
    """Select & order one core's edges for one conv (per-parity segments)."""
    i = np.asarray(indices[0])
    j = np.asarray(indices[1])
    v = np.asarray(values, dtype=np.float32)
    lo = core * NLOC
    sel = (i >= lo) & (i < lo + NLOC)
    il = (i[sel] - lo).astype(np.int64)
    jj = j[sel].astype(np.int64)
    vv = v[sel]
    segs = []
    for par in (0, 1):
        m = (jj & 1) == par
        il2, jj2, vv2 = il[m], jj[m], vv[m]
        n = il2.size
        if n == 0:
            segs.append(dict(jh=np.zeros(0, np.int16), ig=np.zeros(0, np.int16),
                             isc=np.zeros(0, np.int16), v=np.zeros(0, np.float32),
                             gsz=[]))
            continue
        # rank within target (this segment)
        order = np.argsort(il2, kind='stable')
        il2, jj2, vv2 = il2[order], jj2[order], vv2[order]
        change = np.r_[True, il2[1:] != il2[:-1]]
        seg_start = np.flatnonzero(change)
        starts_rep = np.repeat(seg_start, np.diff(np.r_[seg_start, n]))
        rank = np.arange(n) - starts_rep
        # rank-major order (stable keeps target-sorted within a rank)
        order2 = np.argsort(rank, kind='stable')
        il2, jj2, vv2, rank = il2[order2], jj2[order2], vv2[order2], rank[order2]
        gsz = np.bincount(rank).tolist()
        segs.append(dict(
            jh=(jj2 >> 1).astype(np.int16),
            ig=il2.astype(np.int16),
            isc=il2.astype(np.int16),
            v=vv2,
            gsz=gsz,
        ))
    return segs


def _layout_segment(gmax):
    """Uniform stream layout for one (conv, parity) segment from max-over-cores
    rank-group sizes. Returns (padded_group_sizes, total, calls) where calls
    are (start, end, img): one scatter call per rank-group piece, img =
    rank % KIMG selects the accumulator image tensor. Per-call row
    uniqueness holds because a rank group has at most one edge per target."""
    R = len(gmax)
    D = 0
    while D < R and gmax[D] >= DENSE_T:
        D += 1
    padded = [LPAD] * D + [_round128(max(1, g)) for g in gmax[D:]]
    total0 = sum(padded)
    total = (total0 + CH - 1) // CH * CH
    calls = []
    pos = 0
    for r in range(R):
        s, e = pos, pos + padded[r]
        kind = 'd' if r < D else 's'
        p = s
        while p < e:
            q = min(e, (p // CH + 1) * CH)
            calls.append([kind, p, q, r % KIMG])
            p = q
        pos = e
    # trailing pad: merge into last sparse call when same chunk, else own call
    p = pos
    while p < total:
        q = min(total, (p // CH + 1) * CH)
        if calls and calls[-1][0] == 's' and calls[-1][2] == p \
                and (calls[-1][1] // CH) == (p // CH):
            calls[-1][2] = q
        else:
            calls.append(['s', p, q, 0])
        p = q
    return padded, total, [tuple(c) for c in calls], D


def _place_segment(seg, padded, total, D):
    """Scatter one core's segment edges into the padded uniform stream.
    Dense groups (r < D) use slot position = target id; sparse groups pack
    their edges at the group start."""
    jh = np.zeros(total, np.int16)
    ig = np.zeros(total, np.int16)
    isc = np.full(total, GARBAGE, np.int16)
    v = np.zeros(total, np.float32)
    gsz = seg['gsz']
    pos = 0
    off = 0
    for r, p in enumerate(padded):
        g = gsz[r] if r < len(gsz) else 0
        if g:
            sl = slice(off, off + g)
            if r < D:
                tgt = seg['ig'][sl].astype(np.int64)   # targets, sorted
                at = pos + tgt
                jh[at] = seg['jh'][sl]
                ig[at] = seg['ig'][sl]
                isc[at] = seg['isc'][sl]
                v[at] = seg['v'][sl]
            else:
                jh[pos:pos + g] = seg['jh'][sl]
                ig[pos:pos + g] = seg['ig'][sl]
                isc[pos:pos + g] = seg['isc'][sl]
                v[pos:pos + g] = seg['v'][sl]
            off += g
        pos += p
    return jh, ig, isc, v


def _wrap16(arr):
    """[n] -> [128, n/16] int16, slot k = col*16 + row, replicated x8."""
    n = arr.size
    w = arr.reshape(n // 16, 16).T
    return np.tile(w, (8, 1)).copy()


def _wrap128(arr):
    n = arr.size
    return arr.reshape(n // 128, 128).T.copy()


def _host_prep(x, lower_indices, lower_values, upper_indices, upper_values,
               w_lower, a_lower, w_upper, a_upper, w_lin):
    x = np.asarray(x, np.float32)
    w_lower = np.asarray(w_lower, np.float32)
    w_upper = np.asarray(w_upper, np.float32)
    a_lower = np.asarray(a_lower, np.float32)
    a_upper = np.asarray(a_upper, np.float32)
    w_lin = np.asarray(w_lin, np.float32)

    xt = np.zeros((64, RPAD), bfloat16)
    xt[:, :N] = x.T.astype(bfloat16)

    rhs = np.zeros((64, 132), bfloat16)
    rhs[:, 0:64] = w_lower.astype(bfloat16)
    rhs[:, 64] = (w_lower @ a_lower[:64]).astype(bfloat16)
    rhs[:, 65] = (w_lower @ a_lower[64:]).astype(bfloat16)
    rhs[:, 66:130] = w_upper.astype(bfloat16)
    rhs[:, 130] = (w_upper @ a_upper[:64]).astype(bfloat16)
    rhs[:, 131] = (w_upper @ a_upper[64:]).astype(bfloat16)

    rhsloc = np.zeros((64, 2), bfloat16)
    rhsloc[:, 0] = (w_lower @ a_lower[64:]).astype(bfloat16)
    rhsloc[:, 1] = (w_upper @ a_upper[64:]).astype(bfloat16)

    wlin = (w_lin * EPS).astype(bfloat16)

    convs = [(lower_indices, lower_values), (upper_indices, upper_values)]
    per_core = [[_prep_conv_core(ix, vv, c) for (ix, vv) in convs]
                for c in range(NCORES)]

    plans = []   # per conv, per parity: (padded, total, calls)
    for cv in range(2):
        pp = []
        for par in (0, 1):
            rmax = max(len(per_core[c][cv][par]['gsz']) for c in range(NCORES))
            gmax = [max((per_core[c][cv][par]['gsz'][r]
                         if r < len(per_core[c][cv][par]['gsz']) else 0)
                        for c in range(NCORES)) for r in range(rmax)]
            pp.append(_layout_segment(gmax))
        plans.append(pp)

    in_maps = []
    for c in range(NCORES):
        m = {
            'xt': xt,
            'xtloc': np.ascontiguousarray(
                np.pad(x[c * NLOC:(c + 1) * NLOC].T.astype(bfloat16),
                       ((0, 0), (0, LPAD - NLOC)))),
            'rhs': rhs,
            'rhsloc': rhsloc,
            'wlin': wlin,
        }
        for cv, name in ((0, 'l'), (1, 'u')):
            jts, its, sts, vts = [], [], [], []
            for par in (0, 1):
                padded, total, _calls, D = plans[cv][par]
                jh, ig, isc, v = _place_segment(per_core[c][cv][par], padded,
                                                total, D)
                for s in range(0, total, CH):
                    jts.append(_wrap16(jh[s:s + CH]))
                    its.append(_wrap16(ig[s:s + CH]))
                    sts.append(_wrap16(isc[s:s + CH]))
                    vts.append(_wrap128(v[s:s + CH]))
            m[f'jx_{name}'] = np.stack(jts)
            m[f'ix_{name}'] = np.stack(its)
            m[f'sx_{name}'] = np.stack(sts)
            m[f'vx_{name}'] = np.stack(vts)
        in_maps.append(m)
    return in_maps, plans


# ---------------------------------------------------------------- emulation

def _emulate(in_maps, plans):
    """Numpy emulation of the device graph (bf16 where the device is bf16)."""
    outs = []
    f32 = np.float32
    for c in range(NCORES):
        m = in_maps[c]
        xt = m['xt'].astype(f32)          # [64, RPAD]
        rhs = m['rhs'].astype(f32)        # [64, 132]
        tblL = (xt.T @ rhs[:, 0:66]).astype(bfloat16)   # [RPAD, 66]
        tblU = (xt.T @ rhs[:, 66:132]).astype(bfloat16)
        xl = m['xtloc'].astype(f32)       # [64, LPAD]
        sloc = (xl.T @ m['rhsloc'].astype(f32)).astype(bfloat16)  # [LPAD, 2]
        acc = np.zeros((KIMG, ACC_ROWS, 64), f32)
        for cv, name in ((0, 'l'), (1, 'u')):
            tbl = tblL if cv == 0 else tblU
            jx, ix, sx, vx = (m[f'jx_{name}'], m[f'ix_{name}'],
                              m[f'sx_{name}'], m[f'vx_{name}'])
            nch = jx.shape[0]
            chunk_par = []
            chunk_calls = {}
            for par in (0, 1):
                _padded, total, calls, D = plans[cv][par]
                base = 0 if par == 0 else plans[cv][0][1]
                chunk_par += [par] * (total // CH)
                for kind, a, b, img in calls:
                    ga, gb = a + base, b + base
                    toff = (a - (a // LPAD) * LPAD) // 128 if kind == 'd' else 0
                    chunk_calls.setdefault(ga // CH, []).append(
                        (kind, ga % CH, ((gb - 1) % CH) + 1, img, toff))
            for ch in range(nch):
                par = chunk_par[ch]
                jlin = jx[ch][:16].T.reshape(-1).astype(np.int64)
                ilin = ix[ch][:16].T.reshape(-1).astype(np.int64)
                slin = sx[ch][:16].T.reshape(-1).astype(np.int64)
                vlin = vx[ch].T.reshape(-1)
                rows = tbl[2 * jlin + par]                      # [CH, 66] bf16
                sJ = rows[:, 64].astype(f32)
                sI = sloc[ilin, cv].astype(f32)
                z = sJ + sI
                e = np.exp(np.minimum(z, 0.0))
                alpha = ((np.maximum(z, 0.0) + e - 1.0) * vlin).astype(bfloat16)
                msgs = rows[:, 0:64].astype(f32) * alpha.astype(f32)[:, None]
                for kind, a, b, img, toff in chunk_calls.get(ch, []):
                    if kind == 'd':
                        ra = toff * 128
                        acc[img][ra:ra + (b - a)] += msgs[a:b]
                    else:
                        np.add.at(acc[img], slin[a:b], msgs[a:b])
        wx = (m['xtloc'].astype(f32).T @ m['wlin'].astype(f32))  # [LPAD, 64]
        red = acc[:, 0:NLOC].sum(axis=0)
        out = np.maximum(red + wx[:NLOC], 0.0)
        outs.append(out.astype(f32))
    return np.concatenate(outs, axis=0)


# ---------------------------------------------------------------- device

def _build_graph(plans):
    import concourse.bass as bass
    import concourse.bacc as bacc
    import concourse.mybir as mybir
    import concourse.tile as tile

    dt = mybir.dt
    Alu = mybir.AluOpType
    Act = mybir.ActivationFunctionType

    nc = bacc.Bacc(None)

    xt_p = nc.declare_dram_parameter('xt', [64, RPAD], dt.bfloat16, isOutput=False)
    xtloc_p = nc.declare_dram_parameter('xtloc', [64, LPAD], dt.bfloat16, isOutput=False)
    rhs_p = nc.declare_dram_parameter('rhs', [64, 132], dt.bfloat16, isOutput=False)
    rhsloc_p = nc.declare_dram_parameter('rhsloc', [64, 2], dt.bfloat16, isOutput=False)
    wlin_p = nc.declare_dram_parameter('wlin', [64, 64], dt.bfloat16, isOutput=False)
    edge_p = {}
    nch_conv = []
    for cv, name in ((0, 'l'), (1, 'u')):
        nch = (plans[cv][0][1] + plans[cv][1][1]) // CH
        nch_conv.append(nch)
        edge_p[f'jx_{name}'] = nc.declare_dram_parameter(
            f'jx_{name}', [nch, 128, CH // 16], dt.int16, isOutput=False)
        edge_p[f'ix_{name}'] = nc.declare_dram_parameter(
            f'ix_{name}', [nch, 128, CH // 16], dt.int16, isOutput=False)
        edge_p[f'sx_{name}'] = nc.declare_dram_parameter(
            f'sx_{name}', [nch, 128, CH // 16], dt.int16, isOutput=False)
        edge_p[f'vx_{name}'] = nc.declare_dram_parameter(
            f'vx_{name}', [nch, 128, CH // 128], dt.float32, isOutput=False)
    out_p = nc.declare_dram_parameter('out', [LPAD, 64], dt.float32, isOutput=True)

    tbl = [nc.dram_tensor('tblL', [RPAD // 2, 256], dt.bfloat16),
           nc.dram_tensor('tblU', [RPAD // 2, 256], dt.bfloat16)]
    loc = [nc.dram_tensor('locL', [LPAD, 128], dt.bfloat16),
           nc.dram_tensor('locU', [LPAD, 128], dt.bfloat16)]
    acc_imgs = [nc.dram_tensor(f'acc{k}', [ACC_ROWS, 64], dt.float32)
                for k in range(KIMG)]

    with tile.TileContext(nc) as tc:
        with tc.tile_pool(name='keep', bufs=1) as keep:
            xtloc_sb = keep.tile([64, LPAD], dt.bfloat16)
            nc.sync.dma_start(xtloc_sb[:], xtloc_p[:])
            rhs_sb = keep.tile([64, 132], dt.bfloat16)
            nc.sync.dma_start(rhs_sb[:], rhs_p[:])
            rhsloc_sb = keep.tile([64, 2], dt.bfloat16)
            nc.sync.dma_start(rhsloc_sb[:], rhsloc_p[:])
            wlin_sb = keep.tile([64, 64], dt.bfloat16)
            nc.sync.dma_start(wlin_sb[:], wlin_p[:])
            zeros_kc = keep.tile([128, CH // 128], dt.float32)
            nc.vector.memset(zeros_kc[:], 0.0)

            # ------------- phase 1: tables --------------------------------
            with tc.tile_pool(name='xtp', bufs=1) as xtp, \
                 tc.tile_pool(name='zp', bufs=1) as zp, \
                 tc.tile_pool(name='ps', bufs=4, space=bass.MemorySpace.PSUM) as psp, \
                 tc.tile_pool(name='stripe', bufs=3) as stp:
                xt_sb = xtp.tile([64, RPAD], dt.bfloat16)
                nc.sync.dma_start(xt_sb[:], xt_p[:])

                # zero the accumulator images
                zt = zp.tile([128, 3200], dt.float32)
                nc.vector.memset(zt[:], 0.0)
                per = ACC_ROWS * 64 // 128
                for k in range(KIMG):
                    accv = acc_imgs[k][:].flatten().rearrange('(p f) -> p f', p=128)
                    nc.sync.dma_start(accv[:, 0:per], zt[:, 0:per])

                locstripe = keep.tile([128, NTL, 2], dt.bfloat16)
                for t in range(NTL):
                    ps = psp.tile([128, 2], dt.float32, tag='mmloc')
                    nc.tensor.matmul(ps[:], xtloc_sb[:, t * 128:(t + 1) * 128],
                                     rhsloc_sb[:], start=True, stop=True)
                    nc.scalar.activation(locstripe[:, t, :], ps[:], Act.Copy)
                for cv in range(2):
                    dst = loc[cv][:].flatten().rearrange(
                        '(t p c) -> p t c', t=NTL, p=128, c=128)[:, :, 0:1]
                    nc.sync.dma_start(dst, locstripe[:, :, cv:cv + 1])

                GQ = 8
                for g0 in range(0, NT, GQ):
                    ng = min(GQ, NT - g0)
                    stripe = stp.tile([128, GQ, 132], dt.bfloat16, tag='stripe')
                    for g in range(ng):
                        t = g0 + g
                        ps = psp.tile([128, 132], dt.float32, tag='mm')
                        nc.tensor.matmul(ps[:], xt_sb[:, t * 128:(t + 1) * 128],
                                         rhs_sb[:], start=True, stop=True)
                        if t % 2 == 0:
                            nc.scalar.activation(stripe[:, g, :], ps[:], Act.Copy)
                        else:
                            nc.vector.tensor_copy(stripe[:, g, :], ps[:])
                    for cv in range(2):
                        dst = tbl[cv][:].flatten().rearrange(
                            '(t p c) -> p t c', t=NT, p=128, c=128)[:, g0:g0 + ng, 0:66]
                        nc.sync.dma_start(dst, stripe[:, 0:ng, cv * 66:cv * 66 + 66])

            # ------------- phase 3: edges ---------------------------------
            with tc.tile_pool(name='edges', bufs=5) as ep:
                KC = CH // 128
                for cv, name in ((0, 'l'), (1, 'u')):
                    chunk_par = []
                    dense_bc = {}
                    sparse_bc = {}
                    sc0_bc = {}
                    for par in (0, 1):
                        _padded, total, calls, D = plans[cv][par]
                        base = 0 if par == 0 else plans[cv][0][1]
                        nch0 = len(chunk_par)
                        chunk_par += [par] * (total // CH)
                        for kind, a, b, img in calls:
                            ga, gb = a + base, b + base
                            assert ga // CH == (gb - 1) // CH
                            chd = ga // CH
                            c0 = (ga % CH) // 128
                            c1 = ((gb - 1) % CH) // 128 + 1
                            if kind == 'd':
                                r = a // LPAD
                                dense_bc.setdefault(chd, []).append(
                                    (c0, c1, (a - r * LPAD) // 128, img))
                            else:
                                sparse_bc.setdefault(chd, []).append((c0, c1, img))
                        dend = D * LPAD
                        for chd in range(nch0, len(chunk_par)):
                            a0 = (chd - nch0) * CH
                            sc0_bc[chd] = max(0, min(CH, dend - a0)) // 128
                    for ch in range(nch_conv[cv]):
                        par = chunk_par[ch]
                        sc0 = sc0_bc[ch]
                        nsp = KC - sc0
                        jt = ep.tile([128, CH // 16], dt.int16, tag='jt')
                        vt = ep.tile([128, KC], dt.float32, tag='vt')
                        nc.sync.dma_start(jt[:], edge_p[f'jx_{name}'][ch])
                        nc.sync.dma_start(vt[:], edge_p[f'vx_{name}'][ch])

                        gJ = ep.tile([128, KC, 128], dt.bfloat16, tag='gJ')
                        off = par * 128
                        nc.gpsimd.dma_gather(
                            gJ[:], tbl[cv][:, off:off + 128], jt[:],
                            num_idxs=CH, num_idxs_reg=CH, elem_size=128,
                            elem_step=256, single_packet=False)
                        if nsp > 0:
                            it = ep.tile([128, CH // 16], dt.int16, tag='it')
                            st = ep.tile([128, CH // 16], dt.int16, tag='st')
                            nc.sync.dma_start(it[:], edge_p[f'ix_{name}'][ch])
                            nc.sync.dma_start(st[:], edge_p[f'sx_{name}'][ch])
                            gI = ep.tile([128, KC, 128], dt.bfloat16, tag='gI')
                            nc.gpsimd.dma_gather(
                                gI[:, 0:nsp, :], loc[cv][:, 0:128],
                                it[:, sc0 * 8:],
                                num_idxs=nsp * 128, num_idxs_reg=nsp * 128,
                                elem_size=128, elem_step=128,
                                single_packet=False)

                        z = ep.tile([128, KC], dt.float32, tag='z')
                        for (c0, c1, toff, img) in dense_bc.get(ch, []):
                            nc.vector.tensor_tensor(
                                z[:, c0:c1], gJ[:, c0:c1, 64],
                                locstripe[:, toff:toff + (c1 - c0), cv], Alu.add)
                        if nsp > 0:
                            nc.vector.tensor_tensor(
                                z[:, sc0:KC], gJ[:, sc0:KC, 64],
                                gI[:, 0:nsp, 0], Alu.add)
                        ex = ep.tile([128, KC], dt.float32, tag='ex')
                        nc.scalar.activation(ex[:], z[:], Act.Exp)
                        em1 = ep.tile([128, KC], dt.float32, tag='em1')
                        nc.vector.scalar_tensor_tensor(em1[:], ex[:], 1.0,
                                                       zeros_kc[:], Alu.min, Alu.add)
                        t1 = ep.tile([128, KC], dt.float32, tag='t1')
                        nc.vector.scalar_tensor_tensor(t1[:], z[:], 0.0, em1[:],
                                                       Alu.max, Alu.add)
                        alpha = ep.tile([128, KC], dt.bfloat16, tag='alpha')
                        nc.vector.scalar_tensor_tensor(alpha[:], t1[:], -1.0,
                                                       vt[:], Alu.add, Alu.mult)
                        msgs = ep.tile([128, KC, 64], dt.float32, tag='msgs')
                        ab = alpha[:].unsqueeze(2).to_broadcast([128, KC, 64])
                        nc.vector.tensor_tensor(msgs[:], gJ[:, :, 0:64], ab,
                                                Alu.mult)

                        for (c0, c1, toff, img) in dense_bc.get(ch, []):
                            dst = acc_imgs[img][:].flatten()[
                                toff * 128 * 64:(toff + (c1 - c0)) * 128 * 64]
                            nc.gpsimd.dma_start(
                                dst.rearrange('(b p c) -> p b c', p=128, c=64),
                                msgs[:, c0:c1, :], accum_op=Alu.add)
                        for (c0, c1, img) in sparse_bc.get(ch, []):
                            nc.gpsimd.dma_scatter_add(
                                acc_imgs[img][:, :], msgs[:, c0:c1, :],
                                st[:, c0 * 8:c1 * 8],
                                num_idxs=(c1 - c0) * 128,
                                num_idxs_reg=(c1 - c0) * 128, elem_size=64)

            # ------------- phase 4: epilogue ------------------------------
            with tc.tile_pool(name='epi', bufs=3) as pp, \
                 tc.tile_pool(name='ps2', bufs=4, space=bass.MemorySpace.PSUM) as ps2:
                outflat = out_p[:].flatten()
                for t in range(NTL):
                    ps = ps2.tile([128, 64], dt.float32, tag='wx')
                    nc.tensor.matmul(ps[:], xtloc_sb[:, t * 128:(t + 1) * 128],
                                     wlin_sb[:], start=True, stop=True)
                    img = pp.tile([128, KIMG, 64], dt.float32, tag='img')
                    for k in range(KIMG):
                        srcap = acc_imgs[k][:].flatten()[t * 128 * 64:
                                                         (t + 1) * 128 * 64]
                        nc.sync.dma_start(img[:, k, :],
                                          srcap.rearrange('(p c) -> p c', p=128))
                    red = pp.tile([128, 64], dt.float32, tag='red')
                    nc.vector.tensor_tensor(red[:], img[:, 0, :], img[:, 1, :],
                                            Alu.add)
                    nc.vector.tensor_tensor(red[:], red[:], img[:, 2, :], Alu.add)
                    nc.vector.tensor_tensor(red[:], red[:], img[:, 3, :], Alu.add)
                    nc.vector.tensor_tensor(red[:], red[:], img[:, 4, :], Alu.add)
                    nc.vector.tensor_tensor(red[:], red[:], ps[:], Alu.add)
                    ot = pp.tile([128, 64], dt.float32, tag='ot')
                    nc.scalar.activation(ot[:], red[:], Act.Relu)
                    nc.sync.dma_start(
                        outflat[t * 128 * 64:(t + 1) * 128 * 64]
                        .rearrange('(p c) -> p c', p=128), ot[:])

    nc.compile()
    return nc


_cached = {}


def kernel(x, lower_indices, lower_values, upper_indices, upper_values,
           w_lower, a_lower, w_upper, a_upper, w_lin, _emulate_only=False,
           _trace=False):
    from concourse.bass_utils import run_bass_kernel_spmd

    in_maps, plans = _host_prep(
        x, lower_indices, lower_values, upper_indices, upper_values,
        w_lower, a_lower, w_upper, a_upper, w_lin)
    if _emulate_only:
        return _emulate(in_maps, plans)

    key = tuple((plans[cv][par][1], tuple(map(tuple, plans[cv][par][2])))
                for cv in range(2) for par in (0, 1))
    if key not in _cached:
        _cached[key] = _build_graph(plans)
    nc = _cached[key]
    res = run_bass_kernel_spmd(nc, in_maps, core_ids=list(range(NCORES)),
                               trace=_trace)
    out = np.concatenate([res.results[c]['out'][:NLOC] for c in range(NCORES)],
                         axis=0).astype(np.float32)
    kernel._last_exec_ns = res.exec_time_ns
    kernel._last_res = res
    return out



# revision 5
# speedup vs baseline: 1.4782x; 1.4782x over previous
"""CANLayer (two attention convs + linear, relu) on 8 trn2 NeuronCores.

Strategy: shard edges by target-node range (6250 rows/core, no collectives).
Per conv, a node table [xm | s_src | s_tgt] lives in HBM (built on device by
PE); per edge we dma_gather the source row (xm[j], s_src[j]) and the target's
s_tgt from a per-core local table, compute alpha = elu(s_src+s_tgt)*v on
DVE/ACT, scale the message, and dma_scatter_add (CCE f32) into a local
accumulator. The scatter uses 5 rank-images (row = i_loc + 6250*(rank%5)) so
every scatter call has unique rows (the CCE RMW loses updates on duplicate
rows within a call). Epilogue reduces the images, adds x@w_lin*EPS, applies
relu.
"""
import sys
import numpy as np

for _p in ('/opt/trn_rl_repo',):
    if _p not in sys.path:
        sys.path.insert(0, _p)

import ml_dtypes

bfloat16 = ml_dtypes.bfloat16

N = 50000
E = 800000
C = 64
NCORES = 8
NLOC = N // NCORES            # 6250
EPS = 1 + 1e-06

NT = 391                       # table tiles of 128 rows
RPAD = NT * 128                # 50048
NTL = 49                       # local tiles
LPAD = NTL * 128               # 6272
CH = 4096                      # edges per chunk (max per dma_gather call)
KIMG = 5                       # scatter rank images (separate acc tensors)
GARBAGE = LPAD                 # 6272: dedicated garbage row in each image
ACC_ROWS = 6400                # per-image rows (>= GARBAGE+1)
DENSE_T = 4200                 # leading rank groups >= this become dense (one
                               # slot per target; no i-gather, accum-DMA scatter)


def _round128(x):
    return (x + 127) // 128 * 128


def _prep_conv_core(indices, values, core):
    """Select & order one core's edges for one conv (per-parity segments)."""
    i = np.asarray(indices[0])
    j = np.asarray(indices[1])
    v = np.asarray(values, dtype=np.float32)
    lo = core * NLOC
    sel = (i >= lo) & (i < lo + NLOC)
    il = (i[sel] - lo).astype(np.int64)
    jj = j[sel].astype(np.int64)
    vv = v[sel]
    segs = []
    for par in (0, 1):
        m = (jj & 1) == par
        il2, jj2, vv2 = il[m], jj[m], vv[m]
        n = il2.size
        if n == 0:
            segs.append(dict(jh=np.zeros(0, np.int16), ig=np.zeros(0, np.int16),
                             isc=np.zeros(0, np.int16), v=np.zeros(0, np.float32),
                             gsz=[]))
            continue
        # rank within target (this segment)
        order = np.argsort(il2, kind='stable')
        il2, jj2, vv2 = il2[order], jj2[order], vv2[order]
        change = np.r_[True, il2[1:] != il2[:-1]]
        seg_start = np.flatnonzero(change)
        starts_rep = np.repeat(seg_start, np.diff(np.r_[seg_start, n]))
        rank = np.arange(n) - starts_rep
        # rank-major order (stable keeps target-sorted within a rank)
        order2 = np.argsort(rank, kind='stable')
        il2, jj2, vv2, rank = il2[order2], jj2[order2], vv2[order2], rank[order2]
        gsz = np.bincount(rank).tolist()
        segs.append(dict(
            jh=(jj2 >> 1).astype(np.int16),
            ig=il2.astype(np.int16),
            isc=il2.astype(np.int16),
            v=vv2,
            gsz=gsz,
        ))
    return segs


def _layout_segment(gmax):
    """Uniform stream layout for one (conv, parity) segment from max-over-cores
    rank-group sizes. Returns (padded_group_sizes, total, calls) where calls
    are (start, end, img): one scatter call per rank-group piece, img =
    rank % KIMG selects the accumulator image tensor. Per-call row
    uniqueness holds because a rank group has at most one edge per target."""
    R = len(gmax)
    D = 0
    while D < R and gmax[D] >= DENSE_T:
        D += 1
    padded = [LPAD] * D + [_round128(max(1, g)) for g in gmax[D:]]
    total0 = sum(padded)
    total = (total0 + CH - 1) // CH * CH
    calls = []
    pos = 0
    for r in range(R):
        s, e = pos, pos + padded[r]
        kind = 'd' if r < D else 's'
        p = s
        while p < e:
            q = min(e, (p // CH + 1) * CH)
            calls.append([kind, p, q, r % KIMG])
            p = q
        pos = e
    # trailing pad: merge into last sparse call when same chunk, else own call
    p = pos
    while p < total:
        q = min(total, (p // CH + 1) * CH)
        if calls and calls[-1][0] == 's' and calls[-1][2] == p \
                and (calls[-1][1] // CH) == (p // CH):
            calls[-1][2] = q
        else:
            calls.append(['s', p, q, 0])
        p = q
    return padded, total, [tuple(c) for c in calls], D


def _place_segment(seg, padded, total, D):
    """Scatter one core's segment edges into the padded uniform stream.
    Dense groups (r < D) use slot position = target id; sparse groups pack
    their edges at the group start."""
    jh = np.zeros(total, np.int16)
    ig = np.zeros(total, np.int16)
    isc = np.full(total, GARBAGE, np.int16)
    v = np.zeros(total, np.float32)
    gsz = seg['gsz']
    pos = 0
    off = 0
    for r, p in enumerate(padded):
        g = gsz[r] if r < len(gsz) else 0
        if g:
            sl = slice(off, off + g)
            if r < D:
                tgt = seg['ig'][sl].astype(np.int64)   # targets, sorted
                at = pos + tgt
                jh[at] = seg['jh'][sl]
                ig[at] = seg['ig'][sl]
                isc[at] = seg['isc'][sl]
                v[at] = seg['v'][sl]
            else:
                jh[pos:pos + g] = seg['jh'][sl]
                ig[pos:pos + g] = seg['ig'][sl]
                isc[pos:pos + g] = seg['isc'][sl]
                v[pos:pos + g] = seg['v'][sl]
            off += g
        pos += p
    return jh, ig, isc, v


def _wrap16(arr):
    """[n] -> [128, n/16] int16, slot k = col*16 + row, replicated x8."""
    n = arr.size
    w = arr.reshape(n // 16, 16).T
    return np.tile(w, (8, 1)).copy()


def _wrap128(arr):
    n = arr.size
    return arr.reshape(n // 128, 128).T.copy()


def _host_prep(x, lower_indices, lower_values, upper_indices, upper_values,
               w_lower, a_lower, w_upper, a_upper, w_lin):
    x = np.asarray(x, np.float32)
    w_lower = np.asarray(w_lower, np.float32)
    w_upper = np.asarray(w_upper, np.float32)
    a_lower = np.asarray(a_lower, np.float32)
    a_upper = np.asarray(a_upper, np.float32)
    w_lin = np.asarray(w_lin, np.float32)

    xt = np.zeros((64, RPAD), bfloat16)
    xt[:, :N] = x.T.astype(bfloat16)

    rhs = np.zeros((64, 132), bfloat16)
    rhs[:, 0:64] = w_lower.astype(bfloat16)
    rhs[:, 64] = (w_lower @ a_lower[:64]).astype(bfloat16)
    rhs[:, 65] = (w_lower @ a_lower[64:]).astype(bfloat16)
    rhs[:, 66:130] = w_upper.astype(bfloat16)
    rhs[:, 130] = (w_upper @ a_upper[:64]).astype(bfloat16)
    rhs[:, 131] = (w_upper @ a_upper[64:]).astype(bfloat16)

    rhsloc = np.zeros((64, 2), bfloat16)
    rhsloc[:, 0] = (w_lower @ a_lower[64:]).astype(bfloat16)
    rhsloc[:, 1] = (w_upper @ a_upper[64:]).astype(bfloat16)

    wlin = (w_lin * EPS).astype(bfloat16)

    convs = [(lower_indices, lower_values), (upper_indices, upper_values)]
    per_core = [[_prep_conv_core(ix, vv, c) for (ix, vv) in convs]
                for c in range(NCORES)]

    plans = []   # per conv, per parity: (padded, total, calls)
    for cv in range(2):
        pp = []
        for par in (0, 1):
            rmax = max(len(per_core[c][cv][par]['gsz']) for c in range(NCORES))
            gmax = [max((per_core[c][cv][par]['gsz'][r]
                         if r < len(per_core[c][cv][par]['gsz']) else 0)
                        for c in range(NCORES)) for r in range(rmax)]
            pp.append(_layout_segment(gmax))
        plans.append(pp)

    in_maps = []
    for c in range(NCORES):
        m = {
            'xt': xt,
            'xtloc': np.ascontiguousarray(
                np.pad(x[c * NLOC:(c + 1) * NLOC].T.astype(bfloat16),
                       ((0, 0), (0, LPAD - NLOC)))),
            'rhs': rhs,
            'rhsloc': rhsloc,
            'wlin': wlin,
        }
        for cv, name in ((0, 'l'), (1, 'u')):
            jts, its, sts, vts = [], [], [], []
            for par in (0, 1):
                padded, total, _calls, D = plans[cv][par]
                jh, ig, isc, v = _place_segment(per_core[c][cv][par], padded,
                                                total, D)
                for s in range(0, total, CH):
                    jts.append(_wrap16(jh[s:s + CH]))
                    its.append(_wrap16(ig[s:s + CH]))
                    sts.append(_wrap16(isc[s:s + CH]))
                    vts.append(_wrap128(v[s:s + CH]))
            m[f'jx_{name}'] = np.stack(jts)
            m[f'ix_{name}'] = np.stack(its)
            m[f'sx_{name}'] = np.stack(sts)
            m[f'vx_{name}'] = np.stack(vts)
        in_maps.append(m)
    return in_maps, plans


# ---------------------------------------------------------------- emulation

def _emulate(in_maps, plans):
    """Numpy emulation of the device graph (bf16 where the device is bf16)."""
    outs = []
    f32 = np.float32
    for c in range(NCORES):
        m = in_maps[c]
        xt = m['xt'].astype(f32)          # [64, RPAD]
        rhs = m['rhs'].astype(f32)        # [64, 132]
        tblL = (xt.T @ rhs[:, 0:66]).astype(bfloat16)   # [RPAD, 66]
        tblU = (xt.T @ rhs[:, 66:132]).astype(bfloat16)
        xl = m['xtloc'].astype(f32)       # [64, LPAD]
        sloc = (xl.T @ m['rhsloc'].astype(f32)).astype(bfloat16)  # [LPAD, 2]
        acc = np.zeros((KIMG, ACC_ROWS, 64), f32)
        for cv, name in ((0, 'l'), (1, 'u')):
            tbl = tblL if cv == 0 else tblU
            jx, ix, sx, vx = (m[f'jx_{name}'], m[f'ix_{name}'],
                              m[f'sx_{name}'], m[f'vx_{name}'])
            nch = jx.shape[0]
            chunk_par = []
            chunk_calls = {}
            for par in (0, 1):
                _padded, total, calls, D = plans[cv][par]
                base = 0 if par == 0 else plans[cv][0][1]
                chunk_par += [par] * (total // CH)
                for kind, a, b, img in calls:
                    ga, gb = a + base, b + base
                    toff = (a - (a // LPAD) * LPAD) // 128 if kind == 'd' else 0
                    chunk_calls.setdefault(ga // CH, []).append(
                        (kind, ga % CH, ((gb - 1) % CH) + 1, img, toff))
            for ch in range(nch):
                par = chunk_par[ch]
                jlin = jx[ch][:16].T.reshape(-1).astype(np.int64)
                ilin = ix[ch][:16].T.reshape(-1).astype(np.int64)
                slin = sx[ch][:16].T.reshape(-1).astype(np.int64)
                vlin = vx[ch].T.reshape(-1)
                rows = tbl[2 * jlin + par]                      # [CH, 66] bf16
                sJ = rows[:, 64].astype(f32)
                sI = sloc[ilin, cv].astype(f32)
                z = sJ + sI
                e = np.exp(np.minimum(z, 0.0))
                alpha = ((np.maximum(z, 0.0) + e - 1.0) * vlin).astype(bfloat16)
                msgs = rows[:, 0:64].astype(f32) * alpha.astype(f32)[:, None]
                for kind, a, b, img, toff in chunk_calls.get(ch, []):
                    if kind == 'd':
                        ra = toff * 128
                        acc[img][ra:ra + (b - a)] += msgs[a:b]
                    else:
                        np.add.at(acc[img], slin[a:b], msgs[a:b])
        wx = (m['xtloc'].astype(f32).T @ m['wlin'].astype(f32))  # [LPAD, 64]
        red = acc[:, 0:NLOC].sum(axis=0)
        out = np.maximum(red + wx[:NLOC], 0.0)
        outs.append(out.astype(f32))
    return np.concatenate(outs, axis=0)


# ---------------------------------------------------------------- device

def _build_graph(plans):
    import concourse.bass as bass
    import concourse.bacc as bacc
    import concourse.mybir as mybir
    import concourse.tile as tile

    dt = mybir.dt
    Alu = mybir.AluOpType
    Act = mybir.ActivationFunctionType

    nc = bacc.Bacc(None, num_swdge_queues=4)
    _gq = [0]

    def _next_q():
        q = _gq[0]
        _gq[0] = (q + 1) % 4
        return q

    xt_p = nc.declare_dram_parameter('xt', [64, RPAD], dt.bfloat16, isOutput=False)
    xtloc_p = nc.declare_dram_parameter('xtloc', [64, LPAD], dt.bfloat16, isOutput=False)
    rhs_p = nc.declare_dram_parameter('rhs', [64, 132], dt.bfloat16, isOutput=False)
    rhsloc_p = nc.declare_dram_parameter('rhsloc', [64, 2], dt.bfloat16, isOutput=False)
    wlin_p = nc.declare_dram_parameter('wlin', [64, 64], dt.bfloat16, isOutput=False)
    edge_p = {}
    nch_conv = []
    for cv, name in ((0, 'l'), (1, 'u')):
        nch = (plans[cv][0][1] + plans[cv][1][1]) // CH
        nch_conv.append(nch)
        edge_p[f'jx_{name}'] = nc.declare_dram_parameter(
            f'jx_{name}', [nch, 128, CH // 16], dt.int16, isOutput=False)
        edge_p[f'ix_{name}'] = nc.declare_dram_parameter(
            f'ix_{name}', [nch, 128, CH // 16], dt.int16, isOutput=False)
        edge_p[f'sx_{name}'] = nc.declare_dram_parameter(
            f'sx_{name}', [nch, 128, CH // 16], dt.int16, isOutput=False)
        edge_p[f'vx_{name}'] = nc.declare_dram_parameter(
            f'vx_{name}', [nch, 128, CH // 128], dt.float32, isOutput=False)
    out_p = nc.declare_dram_parameter('out', [LPAD, 64], dt.float32, isOutput=True)

    tbl = [nc.dram_tensor('tblL', [RPAD // 2, 256], dt.bfloat16),
           nc.dram_tensor('tblU', [RPAD // 2, 256], dt.bfloat16)]
    loc = [nc.dram_tensor('locL', [LPAD, 128], dt.bfloat16),
           nc.dram_tensor('locU', [LPAD, 128], dt.bfloat16)]
    acc_imgs = [nc.dram_tensor(f'acc{k}', [ACC_ROWS, 64], dt.float32)
                for k in range(KIMG)]

    with tile.TileContext(nc) as tc:
        with tc.tile_pool(name='keep', bufs=1) as keep:
            xtloc_sb = keep.tile([64, LPAD], dt.bfloat16)
            nc.sync.dma_start(xtloc_sb[:], xtloc_p[:])
            rhs_sb = keep.tile([64, 132], dt.bfloat16)
            nc.sync.dma_start(rhs_sb[:], rhs_p[:])
            rhsloc_sb = keep.tile([64, 2], dt.bfloat16)
            nc.sync.dma_start(rhsloc_sb[:], rhsloc_p[:])
            wlin_sb = keep.tile([64, 64], dt.bfloat16)
            nc.sync.dma_start(wlin_sb[:], wlin_p[:])
            zeros_kc = keep.tile([128, CH // 128], dt.float32)
            nc.vector.memset(zeros_kc[:], 0.0)

            # ------------- phase 1: tables --------------------------------
            with tc.tile_pool(name='xtp', bufs=1) as xtp, \
                 tc.tile_pool(name='zp', bufs=1) as zp, \
                 tc.tile_pool(name='ps', bufs=4, space=bass.MemorySpace.PSUM) as psp, \
                 tc.tile_pool(name='stripe', bufs=3) as stp:
                xt_sb = xtp.tile([64, RPAD], dt.bfloat16)
                nc.sync.dma_start(xt_sb[:], xt_p[:])

                # zero the accumulator images
                zt = zp.tile([128, 3200], dt.float32)
                nc.vector.memset(zt[:], 0.0)
                per = ACC_ROWS * 64 // 128
                for k in range(KIMG):
                    accv = acc_imgs[k][:].flatten().rearrange('(p f) -> p f', p=128)
                    nc.sync.dma_start(accv[:, 0:per], zt[:, 0:per])

                locstripe = keep.tile([128, NTL, 2], dt.bfloat16)
                for t in range(NTL):
                    ps = psp.tile([128, 2], dt.float32, tag='mmloc')
                    nc.tensor.matmul(ps[:], xtloc_sb[:, t * 128:(t + 1) * 128],
                                     rhsloc_sb[:], start=True, stop=True)
                    nc.scalar.activation(locstripe[:, t, :], ps[:], Act.Copy)
                for cv in range(2):
                    dst = loc[cv][:].flatten().rearrange(
                        '(t p c) -> p t c', t=NTL, p=128, c=128)[:, :, 0:1]
                    nc.sync.dma_start(dst, locstripe[:, :, cv:cv + 1])

                GQ = 8
                for g0 in range(0, NT, GQ):
                    ng = min(GQ, NT - g0)
                    stripe = stp.tile([128, GQ, 132], dt.bfloat16, tag='stripe')
                    for g in range(ng):
                        t = g0 + g
                        ps = psp.tile([128, 132], dt.float32, tag='mm')
                        nc.tensor.matmul(ps[:], xt_sb[:, t * 128:(t + 1) * 128],
                                         rhs_sb[:], start=True, stop=True)
                        if t % 2 == 0:
                            nc.scalar.activation(stripe[:, g, :], ps[:], Act.Copy)
                        else:
                            nc.vector.tensor_copy(stripe[:, g, :], ps[:])
                    for cv in range(2):
                        dst = tbl[cv][:].flatten().rearrange(
                            '(t p c) -> p t c', t=NT, p=128, c=128)[:, g0:g0 + ng, 0:66]
                        nc.sync.dma_start(dst, stripe[:, 0:ng, cv * 66:cv * 66 + 66])

            # ------------- phase 3: edges ---------------------------------
            with tc.tile_pool(name='edges', bufs=5) as ep:
                KC = CH // 128
                for cv, name in ((0, 'l'), (1, 'u')):
                    chunk_par = []
                    dense_bc = {}
                    sparse_bc = {}
                    sc0_bc = {}
                    for par in (0, 1):
                        _padded, total, calls, D = plans[cv][par]
                        base = 0 if par == 0 else plans[cv][0][1]
                        nch0 = len(chunk_par)
                        chunk_par += [par] * (total // CH)
                        for kind, a, b, img in calls:
                            ga, gb = a + base, b + base
                            assert ga // CH == (gb - 1) // CH
                            chd = ga // CH
                            c0 = (ga % CH) // 128
                            c1 = ((gb - 1) % CH) // 128 + 1
                            if kind == 'd':
                                r = a // LPAD
                                dense_bc.setdefault(chd, []).append(
                                    (c0, c1, (a - r * LPAD) // 128, img))
                            else:
                                sparse_bc.setdefault(chd, []).append((c0, c1, img))
                        dend = D * LPAD
                        for chd in range(nch0, len(chunk_par)):
                            a0 = (chd - nch0) * CH
                            sc0_bc[chd] = max(0, min(CH, dend - a0)) // 128
                    for ch in range(nch_conv[cv]):
                        par = chunk_par[ch]
                        sc0 = sc0_bc[ch]
                        nsp = KC - sc0
                        jt = ep.tile([128, CH // 16], dt.int16, tag='jt')
                        vt = ep.tile([128, KC], dt.float32, tag='vt')
                        nc.sync.dma_start(jt[:], edge_p[f'jx_{name}'][ch])
                        nc.sync.dma_start(vt[:], edge_p[f'vx_{name}'][ch])

                        gJ = ep.tile([128, KC, 128], dt.bfloat16, tag='gJ')
                        off = par * 128
                        nc.gpsimd.dma_gather(
                            gJ[:], tbl[cv][:, off:off + 128], jt[:],
                            num_idxs=CH, num_idxs_reg=CH, elem_size=128,
                            elem_step=256, single_packet=False,
                            queue_num=_next_q())
                        if nsp > 0:
                            it = ep.tile([128, CH // 16], dt.int16, tag='it')
                            st = ep.tile([128, CH // 16], dt.int16, tag='st')
                            nc.sync.dma_start(it[:], edge_p[f'ix_{name}'][ch])
                            nc.sync.dma_start(st[:], edge_p[f'sx_{name}'][ch])
                            gI = ep.tile([128, KC, 128], dt.bfloat16, tag='gI')
                            nc.gpsimd.dma_gather(
                                gI[:, 0:nsp, :], loc[cv][:, 0:128],
                                it[:, sc0 * 8:],
                                num_idxs=nsp * 128, num_idxs_reg=nsp * 128,
                                elem_size=128, elem_step=128,
                                single_packet=False, queue_num=_next_q())

                        z = ep.tile([128, KC], dt.float32, tag='z')
                        for (c0, c1, toff, img) in dense_bc.get(ch, []):
                            nc.vector.tensor_tensor(
                                z[:, c0:c1], gJ[:, c0:c1, 64],
                                locstripe[:, toff:toff + (c1 - c0), cv], Alu.add)
                        if nsp > 0:
                            nc.vector.tensor_tensor(
                                z[:, sc0:KC], gJ[:, sc0:KC, 64],
                                gI[:, 0:nsp, 0], Alu.add)
                        ex = ep.tile([128, KC], dt.float32, tag='ex')
                        nc.scalar.activation(ex[:], z[:], Act.Exp)
                        em1 = ep.tile([128, KC], dt.float32, tag='em1')
                        nc.vector.scalar_tensor_tensor(em1[:], ex[:], 1.0,
                                                       zeros_kc[:], Alu.min, Alu.add)
                        t1 = ep.tile([128, KC], dt.float32, tag='t1')
                        nc.vector.scalar_tensor_tensor(t1[:], z[:], 0.0, em1[:],
                                                       Alu.max, Alu.add)
                        alpha = ep.tile([128, KC], dt.bfloat16, tag='alpha')
                        nc.vector.scalar_tensor_tensor(alpha[:], t1[:], -1.0,
                                                       vt[:], Alu.add, Alu.mult)
                        msgs = ep.tile([128, KC, 64], dt.float32, tag='msgs')
                        ab = alpha[:].unsqueeze(2).to_broadcast([128, KC, 64])
                        nc.vector.tensor_tensor(msgs[:], gJ[:, :, 0:64], ab,
                                                Alu.mult)

                        for (c0, c1, toff, img) in dense_bc.get(ch, []):
                            dst = acc_imgs[img][:].flatten()[
                                toff * 128 * 64:(toff + (c1 - c0)) * 128 * 64]
                            nc.gpsimd.dma_start(
                                dst.rearrange('(b p c) -> p b c', p=128, c=64),
                                msgs[:, c0:c1, :], accum_op=Alu.add)
                        for (c0, c1, img) in sparse_bc.get(ch, []):
                            nc.gpsimd.dma_scatter_add(
                                acc_imgs[img][:, :], msgs[:, c0:c1, :],
                                st[:, c0 * 8:c1 * 8],
                                num_idxs=(c1 - c0) * 128,
                                num_idxs_reg=(c1 - c0) * 128, elem_size=64,
                                queue_num=_next_q())

            # ------------- phase 4: epilogue ------------------------------
            with tc.tile_pool(name='epi', bufs=3) as pp, \
                 tc.tile_pool(name='ps2', bufs=4, space=bass.MemorySpace.PSUM) as ps2:
                outflat = out_p[:].flatten()
                for t in range(NTL):
                    ps = ps2.tile([128, 64], dt.float32, tag='wx')
                    nc.tensor.matmul(ps[:], xtloc_sb[:, t * 128:(t + 1) * 128],
                                     wlin_sb[:], start=True, stop=True)
                    img = pp.tile([128, KIMG, 64], dt.float32, tag='img')
                    for k in range(KIMG):
                        srcap = acc_imgs[k][:].flatten()[t * 128 * 64:
                                                         (t + 1) * 128 * 64]
                        nc.sync.dma_start(img[:, k, :],
                                          srcap.rearrange('(p c) -> p c', p=128))
                    red = pp.tile([128, 64], dt.float32, tag='red')
                    nc.vector.tensor_tensor(red[:], img[:, 0, :], img[:, 1, :],
                                            Alu.add)
                    nc.vector.tensor_tensor(red[:], red[:], img[:, 2, :], Alu.add)
                    nc.vector.tensor_tensor(red[:], red[:], img[:, 3, :], Alu.add)
                    nc.vector.tensor_tensor(red[:], red[:], img[:, 4, :], Alu.add)
                    nc.vector.tensor_tensor(red[:], red[:], ps[:], Alu.add)
                    ot = pp.tile([128, 64], dt.float32, tag='ot')
                    nc.scalar.activation(ot[:], red[:], Act.Relu)
                    nc.sync.dma_start(
                        outflat[t * 128 * 64:(t + 1) * 128 * 64]
                        .rearrange('(p c) -> p c', p=128), ot[:])

    nc.compile()
    return nc


_cached = {}


def kernel(x, lower_indices, lower_values, upper_indices, upper_values,
           w_lower, a_lower, w_upper, a_upper, w_lin, _emulate_only=False,
           _trace=False):
    from concourse.bass_utils import run_bass_kernel_spmd

    in_maps, plans = _host_prep(
        x, lower_indices, lower_values, upper_indices, upper_values,
        w_lower, a_lower, w_upper, a_upper, w_lin)
    if _emulate_only:
        return _emulate(in_maps, plans)

    key = tuple((plans[cv][par][1], tuple(map(tuple, plans[cv][par][2])))
                for cv in range(2) for par in (0, 1))
    if key not in _cached:
        _cached[key] = _build_graph(plans)
    nc = _cached[key]
    res = run_bass_kernel_spmd(nc, in_maps, core_ids=list(range(NCORES)),
                               trace=_trace)
    out = np.concatenate([res.results[c]['out'][:NLOC] for c in range(NCORES)],
                         axis=0).astype(np.float32)
    kernel._last_exec_ns = res.exec_time_ns
    kernel._last_res = res
    return out



# revision 7
# speedup vs baseline: 2.2356x; 1.5124x over previous
"""CANLayer (two attention convs + linear, relu) on 8 trn2 NeuronCores.

Strategy: shard edges by target-node range (6250 rows/core, no collectives).
Per conv, a node table [xm | s_src | s_tgt] lives in HBM (built on device by
PE); per edge we dma_gather the source row (xm[j], s_src[j]) and the target's
s_tgt from a per-core local table, compute alpha = elu(s_src+s_tgt)*v on
DVE/ACT, scale the message, and dma_scatter_add (CCE f32) into a local
accumulator. The scatter uses 5 rank-images (row = i_loc + 6250*(rank%5)) so
every scatter call has unique rows (the CCE RMW loses updates on duplicate
rows within a call). Epilogue reduces the images, adds x@w_lin*EPS, applies
relu.
"""
import sys
import numpy as np

for _p in ('/opt/trn_rl_repo',):
    if _p not in sys.path:
        sys.path.insert(0, _p)

import ml_dtypes

bfloat16 = ml_dtypes.bfloat16

N = 50000
E = 800000
C = 64
NCORES = 8
NLOC = N // NCORES            # 6250
EPS = 1 + 1e-06

NT = 391                       # table tiles of 128 rows
RPAD = NT * 128                # 50048
NTL = 49                       # local tiles
LPAD = NTL * 128               # 6272
CH = 4096                      # edges per chunk (max per dma_gather call)
KIMG = 5                       # scatter rank images (separate acc tensors)
GARBAGE = LPAD                 # 6272: dedicated garbage row in each image
ACC_ROWS = 6400                # per-image rows (>= GARBAGE+1)
DENSE_T = 4200                 # leading rank groups >= this become dense (one
                               # slot per target; no i-gather, accum-DMA scatter)


def _round128(x):
    return (x + 127) // 128 * 128


def _prep_conv_core(indices, values, core):
    """Select & order one core's edges for one conv (per-parity segments)."""
    i = np.asarray(indices[0])
    j = np.asarray(indices[1])
    v = np.asarray(values, dtype=np.float32)
    lo = core * NLOC
    sel = (i >= lo) & (i < lo + NLOC)
    il = (i[sel] - lo).astype(np.int64)
    jj = j[sel].astype(np.int64)
    vv = v[sel]
    segs = []
    for par in (0, 1):
        m = (jj & 1) == par
        il2, jj2, vv2 = il[m], jj[m], vv[m]
        n = il2.size
        if n == 0:
            segs.append(dict(jh=np.zeros(0, np.int16), ig=np.zeros(0, np.int16),
                             isc=np.zeros(0, np.int16), v=np.zeros(0, np.float32),
                             gsz=[]))
            continue
        # rank within target (this segment)
        order = np.argsort(il2, kind='stable')
        il2, jj2, vv2 = il2[order], jj2[order], vv2[order]
        change = np.r_[True, il2[1:] != il2[:-1]]
        seg_start = np.flatnonzero(change)
        starts_rep = np.repeat(seg_start, np.diff(np.r_[seg_start, n]))
        rank = np.arange(n) - starts_rep
        # rank-major order (stable keeps target-sorted within a rank)
        order2 = np.argsort(rank, kind='stable')
        il2, jj2, vv2, rank = il2[order2], jj2[order2], vv2[order2], rank[order2]
        gsz = np.bincount(rank).tolist()
        segs.append(dict(
            jh=(jj2 >> 1).astype(np.int16),
            ig=il2.astype(np.int16),
            isc=il2.astype(np.int16),
            v=vv2,
            gsz=gsz,
        ))
    return segs


def _layout_segment(gmax):
    """Uniform stream layout for one (conv, parity) segment from max-over-cores
    rank-group sizes. Returns (padded_group_sizes, total, calls) where calls
    are (start, end, img): one scatter call per rank-group piece, img =
    rank % KIMG selects the accumulator image tensor. Per-call row
    uniqueness holds because a rank group has at most one edge per target."""
    R = len(gmax)
    D = 0
    while D < R and gmax[D] >= DENSE_T:
        D += 1
    padded = [LPAD] * D + [_round128(max(1, g)) for g in gmax[D:]]
    total0 = sum(padded)
    total = (total0 + CH - 1) // CH * CH
    calls = []
    pos = 0
    for r in range(R):
        s, e = pos, pos + padded[r]
        kind = 'd' if r < D else 's'
        p = s
        while p < e:
            q = min(e, (p // CH + 1) * CH)
            calls.append([kind, p, q, r % KIMG])
            p = q
        pos = e
    # trailing pad: merge into last sparse call when same chunk, else own call
    p = pos
    while p < total:
        q = min(total, (p // CH + 1) * CH)
        if calls and calls[-1][0] == 's' and calls[-1][2] == p \
                and (calls[-1][1] // CH) == (p // CH):
            calls[-1][2] = q
        else:
            calls.append(['s', p, q, 0])
        p = q
    return padded, total, [tuple(c) for c in calls], D


def _place_segment(seg, padded, total, D):
    """Scatter one core's segment edges into the padded uniform stream.
    Dense groups (r < D) use slot position = target id; sparse groups pack
    their edges at the group start."""
    jh = np.zeros(total, np.int16)
    ig = np.zeros(total, np.int16)
    isc = np.full(total, GARBAGE, np.int16)
    v = np.zeros(total, np.float32)
    gsz = seg['gsz']
    pos = 0
    off = 0
    for r, p in enumerate(padded):
        g = gsz[r] if r < len(gsz) else 0
        if g:
            sl = slice(off, off + g)
            if r < D:
                tgt = seg['ig'][sl].astype(np.int64)   # targets, sorted
                at = pos + tgt
                jh[at] = seg['jh'][sl]
                ig[at] = seg['ig'][sl]
                isc[at] = seg['isc'][sl]
                v[at] = seg['v'][sl]
            else:
                jh[pos:pos + g] = seg['jh'][sl]
                ig[pos:pos + g] = seg['ig'][sl]
                isc[pos:pos + g] = seg['isc'][sl]
                v[pos:pos + g] = seg['v'][sl]
            off += g
        pos += p
    return jh, ig, isc, v


def _wrap16(arr):
    """[n] -> [128, n/16] int16, slot k = col*16 + row, replicated x8."""
    n = arr.size
    w = arr.reshape(n // 16, 16).T
    return np.tile(w, (8, 1)).copy()


def _wrap128(arr):
    n = arr.size
    return arr.reshape(n // 128, 128).T.copy()


def _host_prep(x, lower_indices, lower_values, upper_indices, upper_values,
               w_lower, a_lower, w_upper, a_upper, w_lin):
    x = np.asarray(x, np.float32)
    w_lower = np.asarray(w_lower, np.float32)
    w_upper = np.asarray(w_upper, np.float32)
    a_lower = np.asarray(a_lower, np.float32)
    a_upper = np.asarray(a_upper, np.float32)
    w_lin = np.asarray(w_lin, np.float32)

    xt = np.zeros((64, RPAD), bfloat16)
    xt[:, :N] = x.T.astype(bfloat16)

    rhs = np.zeros((64, 132), bfloat16)
    rhs[:, 0:64] = w_lower.astype(bfloat16)
    rhs[:, 64] = (w_lower @ a_lower[:64]).astype(bfloat16)
    rhs[:, 65] = (w_lower @ a_lower[64:]).astype(bfloat16)
    rhs[:, 66:130] = w_upper.astype(bfloat16)
    rhs[:, 130] = (w_upper @ a_upper[:64]).astype(bfloat16)
    rhs[:, 131] = (w_upper @ a_upper[64:]).astype(bfloat16)

    rhsloc = np.zeros((64, 2), bfloat16)
    rhsloc[:, 0] = (w_lower @ a_lower[64:]).astype(bfloat16)
    rhsloc[:, 1] = (w_upper @ a_upper[64:]).astype(bfloat16)

    wlin = (w_lin * EPS).astype(bfloat16)

    convs = [(lower_indices, lower_values), (upper_indices, upper_values)]
    per_core = [[_prep_conv_core(ix, vv, c) for (ix, vv) in convs]
                for c in range(NCORES)]

    plans = []   # per conv, per parity: (padded, total, calls)
    for cv in range(2):
        pp = []
        for par in (0, 1):
            rmax = max(len(per_core[c][cv][par]['gsz']) for c in range(NCORES))
            gmax = [max((per_core[c][cv][par]['gsz'][r]
                         if r < len(per_core[c][cv][par]['gsz']) else 0)
                        for c in range(NCORES)) for r in range(rmax)]
            pp.append(_layout_segment(gmax))
        plans.append(pp)

    in_maps = []
    for c in range(NCORES):
        m = {
            'xt': xt,
            'xtloc': np.ascontiguousarray(
                np.pad(x[c * NLOC:(c + 1) * NLOC].T.astype(bfloat16),
                       ((0, 0), (0, LPAD - NLOC)))),
            'rhs': rhs,
            'rhsloc': rhsloc,
            'wlin': wlin,
        }
        for cv, name in ((0, 'l'), (1, 'u')):
            jts, its, sts, vts = [], [], [], []
            for par in (0, 1):
                padded, total, _calls, D = plans[cv][par]
                jh, ig, isc, v = _place_segment(per_core[c][cv][par], padded,
                                                total, D)
                for s in range(0, total, CH):
                    jts.append(_wrap16(jh[s:s + CH]))
                    its.append(_wrap16(ig[s:s + CH]))
                    sts.append(_wrap16(isc[s:s + CH]))
                    vts.append(_wrap128(v[s:s + CH]))
            m[f'jx_{name}'] = np.stack(jts)
            m[f'ix_{name}'] = np.stack(its)
            m[f'sx_{name}'] = np.stack(sts)
            m[f'vx_{name}'] = np.stack(vts)
        in_maps.append(m)
    return in_maps, plans


# ---------------------------------------------------------------- emulation

def _emulate(in_maps, plans):
    """Numpy emulation of the device graph (bf16 where the device is bf16)."""
    outs = []
    f32 = np.float32
    for c in range(NCORES):
        m = in_maps[c]
        xt = m['xt'].astype(f32)          # [64, RPAD]
        rhs = m['rhs'].astype(f32)        # [64, 132]
        tblL = (xt.T @ rhs[:, 0:66]).astype(bfloat16)   # [RPAD, 66]
        tblU = (xt.T @ rhs[:, 66:132]).astype(bfloat16)
        xl = m['xtloc'].astype(f32)       # [64, LPAD]
        sloc = (xl.T @ m['rhsloc'].astype(f32)).astype(bfloat16)  # [LPAD, 2]
        acc = np.zeros((KIMG, ACC_ROWS, 64), f32)
        for cv, name in ((0, 'l'), (1, 'u')):
            tbl = tblL if cv == 0 else tblU
            jx, ix, sx, vx = (m[f'jx_{name}'], m[f'ix_{name}'],
                              m[f'sx_{name}'], m[f'vx_{name}'])
            nch = jx.shape[0]
            chunk_par = []
            chunk_calls = {}
            for par in (0, 1):
                _padded, total, calls, D = plans[cv][par]
                base = 0 if par == 0 else plans[cv][0][1]
                chunk_par += [par] * (total // CH)
                for kind, a, b, img in calls:
                    ga, gb = a + base, b + base
                    toff = (a - (a // LPAD) * LPAD) // 128 if kind == 'd' else 0
                    chunk_calls.setdefault(ga // CH, []).append(
                        (kind, ga % CH, ((gb - 1) % CH) + 1, img, toff))
            for ch in range(nch):
                par = chunk_par[ch]
                jlin = jx[ch][:16].T.reshape(-1).astype(np.int64)
                ilin = ix[ch][:16].T.reshape(-1).astype(np.int64)
                slin = sx[ch][:16].T.reshape(-1).astype(np.int64)
                vlin = vx[ch].T.reshape(-1)
                rows = tbl[2 * jlin + par]                      # [CH, 66] bf16
                sJ = rows[:, 64].astype(f32)
                sI = sloc[ilin, cv].astype(f32)
                z = sJ + sI
                e = np.exp(np.minimum(z, 0.0))
                alpha = ((np.maximum(z, 0.0) + e - 1.0) * vlin).astype(bfloat16)
                msgs = rows[:, 0:64].astype(f32) * alpha.astype(f32)[:, None]
                for kind, a, b, img, toff in chunk_calls.get(ch, []):
                    if kind == 'd':
                        ra = toff * 128
                        acc[img][ra:ra + (b - a)] += msgs[a:b]
                    else:
                        np.add.at(acc[img], slin[a:b], msgs[a:b])
        wx = (m['xtloc'].astype(f32).T @ m['wlin'].astype(f32))  # [LPAD, 64]
        red = acc[:, 0:NLOC].sum(axis=0)
        out = np.maximum(red + wx[:NLOC], 0.0)
        outs.append(out.astype(f32))
    return np.concatenate(outs, axis=0)


# ---------------------------------------------------------------- device

def _build_graph(plans):
    import concourse.bass as bass
    import concourse.bacc as bacc
    import concourse.mybir as mybir
    import concourse.tile as tile

    dt = mybir.dt
    Alu = mybir.AluOpType
    Act = mybir.ActivationFunctionType

    nc = bacc.Bacc(None, num_swdge_queues=4)
    _gq = [0]

    def _next_q():
        q = _gq[0]
        _gq[0] = (q + 1) % 4
        return q

    xt_p = nc.declare_dram_parameter('xt', [64, RPAD], dt.bfloat16, isOutput=False)
    xtloc_p = nc.declare_dram_parameter('xtloc', [64, LPAD], dt.bfloat16, isOutput=False)
    rhs_p = nc.declare_dram_parameter('rhs', [64, 132], dt.bfloat16, isOutput=False)
    rhsloc_p = nc.declare_dram_parameter('rhsloc', [64, 2], dt.bfloat16, isOutput=False)
    wlin_p = nc.declare_dram_parameter('wlin', [64, 64], dt.bfloat16, isOutput=False)
    edge_p = {}
    nch_conv = []
    for cv, name in ((0, 'l'), (1, 'u')):
        nch = (plans[cv][0][1] + plans[cv][1][1]) // CH
        nch_conv.append(nch)
        edge_p[f'jx_{name}'] = nc.declare_dram_parameter(
            f'jx_{name}', [nch, 128, CH // 16], dt.int16, isOutput=False)
        edge_p[f'ix_{name}'] = nc.declare_dram_parameter(
            f'ix_{name}', [nch, 128, CH // 16], dt.int16, isOutput=False)
        edge_p[f'sx_{name}'] = nc.declare_dram_parameter(
            f'sx_{name}', [nch, 128, CH // 16], dt.int16, isOutput=False)
        edge_p[f'vx_{name}'] = nc.declare_dram_parameter(
            f'vx_{name}', [nch, 128, CH // 128], dt.float32, isOutput=False)
    out_p = nc.declare_dram_parameter('out', [LPAD, 64], dt.float32, isOutput=True)

    tbl = [nc.dram_tensor('tblL', [RPAD // 2, 256], dt.bfloat16),
           nc.dram_tensor('tblU', [RPAD // 2, 256], dt.bfloat16)]
    loc = [nc.dram_tensor('locL', [LPAD, 128], dt.bfloat16),
           nc.dram_tensor('locU', [LPAD, 128], dt.bfloat16)]
    acc_imgs = [nc.dram_tensor(f'acc{k}', [ACC_ROWS, 64], dt.float32)
                for k in range(KIMG)]

    with tile.TileContext(nc) as tc:
        with tc.tile_pool(name='keep', bufs=1) as keep:
            xtloc_sb = keep.tile([64, LPAD], dt.bfloat16)
            nc.sync.dma_start(xtloc_sb[:], xtloc_p[:])
            rhs_sb = keep.tile([64, 132], dt.bfloat16)
            nc.sync.dma_start(rhs_sb[:], rhs_p[:])
            rhsloc_sb = keep.tile([64, 2], dt.bfloat16)
            nc.sync.dma_start(rhsloc_sb[:], rhsloc_p[:])
            wlin_sb = keep.tile([64, 64], dt.bfloat16)
            nc.sync.dma_start(wlin_sb[:], wlin_p[:])
            zeros_kc = keep.tile([128, CH // 128], dt.float32)
            nc.vector.memset(zeros_kc[:], 0.0)

            # ------------- phase 1: tables --------------------------------
            with tc.tile_pool(name='xtp', bufs=1) as xtp, \
                 tc.tile_pool(name='zp', bufs=1) as zp, \
                 tc.tile_pool(name='ps', bufs=4, space=bass.MemorySpace.PSUM) as psp, \
                 tc.tile_pool(name='stripe', bufs=3) as stp:
                xt_sb = xtp.tile([64, RPAD], dt.bfloat16)
                nc.sync.dma_start(xt_sb[:], xt_p[:])

                # zero the accumulator images
                zt = zp.tile([128, 3200], dt.float32)
                nc.vector.memset(zt[:], 0.0)
                per = ACC_ROWS * 64 // 128
                for k in range(KIMG):
                    accv = acc_imgs[k][:].flatten().rearrange('(p f) -> p f', p=128)
                    nc.sync.dma_start(accv[:, 0:per], zt[:, 0:per])

                locstripe = keep.tile([128, NTL, 2], dt.bfloat16)
                for t in range(NTL):
                    ps = psp.tile([128, 2], dt.float32, tag='mmloc')
                    nc.tensor.matmul(ps[:], xtloc_sb[:, t * 128:(t + 1) * 128],
                                     rhsloc_sb[:], start=True, stop=True)
                    nc.scalar.activation(locstripe[:, t, :], ps[:], Act.Copy)
                for cv in range(2):
                    dst = loc[cv][:].flatten().rearrange(
                        '(t p c) -> p t c', t=NTL, p=128, c=128)[:, :, 0:1]
                    nc.sync.dma_start(dst, locstripe[:, :, cv:cv + 1])

                GQ = 8
                for g0 in range(0, NT, GQ):
                    ng = min(GQ, NT - g0)
                    stripe = stp.tile([128, GQ, 132], dt.bfloat16, tag='stripe')
                    for g in range(ng):
                        t = g0 + g
                        ps = psp.tile([128, 132], dt.float32, tag='mm')
                        nc.tensor.matmul(ps[:], xt_sb[:, t * 128:(t + 1) * 128],
                                         rhs_sb[:], start=True, stop=True)
                        if t % 2 == 0:
                            nc.scalar.activation(stripe[:, g, :], ps[:], Act.Copy)
                        else:
                            nc.vector.tensor_copy(stripe[:, g, :], ps[:])
                    for cv in range(2):
                        dst = tbl[cv][:].flatten().rearrange(
                            '(t p c) -> p t c', t=NT, p=128, c=128)[:, g0:g0 + ng, 0:66]
                        nc.sync.dma_start(dst, stripe[:, 0:ng, cv * 66:cv * 66 + 66])

            # ------------- phase 3: edges ---------------------------------
            with tc.tile_pool(name='edges', bufs=5) as ep:
                KC = CH // 128
                for cv, name in ((0, 'l'), (1, 'u')):
                    chunk_par = []
                    dense_bc = {}
                    sparse_bc = {}
                    sc0_bc = {}
                    for par in (0, 1):
                        _padded, total, calls, D = plans[cv][par]
                        base = 0 if par == 0 else plans[cv][0][1]
                        nch0 = len(chunk_par)
                        chunk_par += [par] * (total // CH)
                        for kind, a, b, img in calls:
                            ga, gb = a + base, b + base
                            assert ga // CH == (gb - 1) // CH
                            chd = ga // CH
                            c0 = (ga % CH) // 128
                            c1 = ((gb - 1) % CH) // 128 + 1
                            if kind == 'd':
                                r = a // LPAD
                                dense_bc.setdefault(chd, []).append(
                                    (c0, c1, (a - r * LPAD) // 128, img))
                            else:
                                sparse_bc.setdefault(chd, []).append((c0, c1, img))
                        dend = D * LPAD
                        for chd in range(nch0, len(chunk_par)):
                            a0 = (chd - nch0) * CH
                            sc0_bc[chd] = max(0, min(CH, dend - a0)) // 128
                    for ch in range(nch_conv[cv]):
                        par = chunk_par[ch]
                        sc0 = sc0_bc[ch]
                        nsp = KC - sc0
                        jt = ep.tile([128, CH // 16], dt.int16, tag='jt')
                        vt = ep.tile([128, KC], dt.float32, tag='vt')
                        nc.sync.dma_start(jt[:], edge_p[f'jx_{name}'][ch])
                        nc.sync.dma_start(vt[:], edge_p[f'vx_{name}'][ch])

                        gJf = ep.tile([128, KC, 256], dt.bfloat16, tag='gJ')
                        off = par * 128
                        nc.gpsimd.dma_gather(
                            gJf[:], tbl[cv][:, 0:256], jt[:],
                            num_idxs=CH, num_idxs_reg=CH, elem_size=256,
                            elem_step=256, single_packet=False,
                            queue_num=_next_q())
                        gJ = gJf[:, :, off:off + 128]
                        if nsp > 0:
                            it = ep.tile([128, CH // 16], dt.int16, tag='it')
                            st = ep.tile([128, CH // 16], dt.int16, tag='st')
                            nc.sync.dma_start(it[:], edge_p[f'ix_{name}'][ch])
                            nc.sync.dma_start(st[:], edge_p[f'sx_{name}'][ch])
                            gI = ep.tile([128, KC, 128], dt.bfloat16, tag='gI')
                            nc.gpsimd.dma_gather(
                                gI[:, 0:nsp, :], loc[cv][:, 0:128],
                                it[:, sc0 * 8:],
                                num_idxs=nsp * 128, num_idxs_reg=nsp * 128,
                                elem_size=128, elem_step=128,
                                single_packet=False, queue_num=_next_q())

                        z = ep.tile([128, KC], dt.float32, tag='z')
                        for (c0, c1, toff, img) in dense_bc.get(ch, []):
                            nc.vector.tensor_tensor(
                                z[:, c0:c1], gJf[:, c0:c1, off + 64],
                                locstripe[:, toff:toff + (c1 - c0), cv], Alu.add)
                        if nsp > 0:
                            nc.vector.tensor_tensor(
                                z[:, sc0:KC], gJf[:, sc0:KC, off + 64],
                                gI[:, 0:nsp, 0], Alu.add)
                        ex = ep.tile([128, KC], dt.float32, tag='ex')
                        nc.scalar.activation(ex[:], z[:], Act.Exp)
                        em1 = ep.tile([128, KC], dt.float32, tag='em1')
                        nc.vector.scalar_tensor_tensor(em1[:], ex[:], 1.0,
                                                       zeros_kc[:], Alu.min, Alu.add)
                        t1 = ep.tile([128, KC], dt.float32, tag='t1')
                        nc.vector.scalar_tensor_tensor(t1[:], z[:], 0.0, em1[:],
                                                       Alu.max, Alu.add)
                        alpha = ep.tile([128, KC], dt.bfloat16, tag='alpha')
                        nc.vector.scalar_tensor_tensor(alpha[:], t1[:], -1.0,
                                                       vt[:], Alu.add, Alu.mult)
                        msgs = ep.tile([128, KC, 64], dt.float32, tag='msgs')
                        ab = alpha[:].unsqueeze(2).to_broadcast([128, KC, 64])
                        nc.vector.tensor_tensor(msgs[:], gJf[:, :, off:off + 64], ab,
                                                Alu.mult)

                        for (c0, c1, toff, img) in dense_bc.get(ch, []):
                            dst = acc_imgs[img][:].flatten()[
                                toff * 128 * 64:(toff + (c1 - c0)) * 128 * 64]
                            nc.gpsimd.dma_start(
                                dst.rearrange('(b p c) -> p b c', p=128, c=64),
                                msgs[:, c0:c1, :], accum_op=Alu.add)
                        for (c0, c1, img) in sparse_bc.get(ch, []):
                            nc.gpsimd.dma_scatter_add(
                                acc_imgs[img][:, :], msgs[:, c0:c1, :],
                                st[:, c0 * 8:c1 * 8],
                                num_idxs=(c1 - c0) * 128,
                                num_idxs_reg=(c1 - c0) * 128, elem_size=64,
                                queue_num=_next_q())

            # ------------- phase 4: epilogue ------------------------------
            with tc.tile_pool(name='epi', bufs=3) as pp, \
                 tc.tile_pool(name='ps2', bufs=4, space=bass.MemorySpace.PSUM) as ps2:
                outflat = out_p[:].flatten()
                for t in range(NTL):
                    ps = ps2.tile([128, 64], dt.float32, tag='wx')
                    nc.tensor.matmul(ps[:], xtloc_sb[:, t * 128:(t + 1) * 128],
                                     wlin_sb[:], start=True, stop=True)
                    img = pp.tile([128, KIMG, 64], dt.float32, tag='img')
                    for k in range(KIMG):
                        srcap = acc_imgs[k][:].flatten()[t * 128 * 64:
                                                         (t + 1) * 128 * 64]
                        nc.sync.dma_start(img[:, k, :],
                                          srcap.rearrange('(p c) -> p c', p=128))
                    red = pp.tile([128, 64], dt.float32, tag='red')
                    nc.vector.tensor_tensor(red[:], img[:, 0, :], img[:, 1, :],
                                            Alu.add)
                    nc.vector.tensor_tensor(red[:], red[:], img[:, 2, :], Alu.add)
                    nc.vector.tensor_tensor(red[:], red[:], img[:, 3, :], Alu.add)
                    nc.vector.tensor_tensor(red[:], red[:], img[:, 4, :], Alu.add)
                    nc.vector.tensor_tensor(red[:], red[:], ps[:], Alu.add)
                    ot = pp.tile([128, 64], dt.float32, tag='ot')
                    nc.scalar.activation(ot[:], red[:], Act.Relu)
                    nc.sync.dma_start(
                        outflat[t * 128 * 64:(t + 1) * 128 * 64]
                        .rearrange('(p c) -> p c', p=128), ot[:])

    nc.compile()
    return nc


_cached = {}


def kernel(x, lower_indices, lower_values, upper_indices, upper_values,
           w_lower, a_lower, w_upper, a_upper, w_lin, _emulate_only=False,
           _trace=False):
    from concourse.bass_utils import run_bass_kernel_spmd

    in_maps, plans = _host_prep(
        x, lower_indices, lower_values, upper_indices, upper_values,
        w_lower, a_lower, w_upper, a_upper, w_lin)
    if _emulate_only:
        return _emulate(in_maps, plans)

    key = tuple((plans[cv][par][1], tuple(map(tuple, plans[cv][par][2])))
                for cv in range(2) for par in (0, 1))
    if key not in _cached:
        _cached[key] = _build_graph(plans)
    nc = _cached[key]
    res = run_bass_kernel_spmd(nc, in_maps, core_ids=list(range(NCORES)),
                               trace=_trace)
    out = np.concatenate([res.results[c]['out'][:NLOC] for c in range(NCORES)],
                         axis=0).astype(np.float32)
    kernel._last_exec_ns = res.exec_time_ns
    kernel._last_res = res
    return out

